# revision 22
# baseline (speedup 1.0000x reference)
"""Causal self-attention on 8 TRN2 NeuronCores.

Sharding: core c handles batch b = c//2 and head-group g = c%2 (8 of 16 heads).
Each core computes its partial y^T = w_proj[slice].T @ o^T (contraction over its
512 o-channels); the host sums the two partials per batch and adds b_proj.

Shapes (hardcoded): B=4, T=2048, C=1024, H=16, HD=64.

All matmul operands are bf16 (x/w_qkv/w_proj cast on host); accumulation is
fp32 in PSUM. x^T is loaded straight from DRAM with the xbar transpose DMA
(issues split across the SP and ACT queues; weight DMAs issued first).
o stays in SBUF (bf16) and feeds proj directly.

Schedule: attention is ACT(exp)-bound, so qkv/v/proj work is emitted in
half-unit (4-matmul) chunks interleaved between attention chunks, keeping the
PE stream dense while ACT crunches exp without starving its 2-deep score
backlog. proj for window m runs inside pair 3 right after (3, m) completes.
Diagonal causal masking is a DVE multiply with a tril mask (gpsimd
affine_select is broken for bf16 on HW, and gpsimd cannot read PSUM).

PSUM (8 banks): ps_main 2x[128,1024] holds score tiles AND filler accumulators
(split filler halves interleave 1:1 with score allocs so rotation deps always
point backward); ps_pv 2x[128,1024] holds the per-window PV accumulator — both
heads side by side, so one reciprocal-normalize chain covers the window.
reciprocal_approx_fast needs its input at partition offset 0 (HW bug), hence
the denominator row is first copied to a [1,1024] sbuf tile.
"""

import numpy as np

B, T, C, H = 4, 2048, 1024, 16
HD = C // H          # 64
G = 2                # head groups
NHL = H // G         # 8 heads per core
GQ = NHL * HD        # 512 channel slice per core
P = 128
NT = T // P          # 16 token tiles / k-chunks
NCHUNK = C // P      # 8 contraction chunks for qkv
SCALE = 1.0 / float(np.sqrt(HD))

_PROGRAM = None


def _emit(ctx, tc, aps, mybir, bass):
    nc = tc.nc
    f32 = mybir.dt.float32
    bf16 = mybir.dt.bfloat16
    EXP = mybir.ActivationFunctionType.Exp

    x_d, wqkv_d, bqk_d, bv_d, wp_d, yT_d = (
        aps["x"], aps["wqkv"], aps["bqk"], aps["bv"], aps["wp"], aps["yT"],
    )

    # ---------------- pools ----------------
    const = ctx.enter_context(tc.tile_pool(name="const", bufs=1))
    ps_main = ctx.enter_context(tc.tile_pool(name="ps_main", bufs=2, space="PSUM"))
    ps_pv = ctx.enter_context(tc.tile_pool(name="ps_pv", bufs=1, space="PSUM"))
    ps_fill = ctx.enter_context(tc.tile_pool(name="ps_fill", bufs=2, space="PSUM"))

    qkp = ctx.enter_context(tc.tile_pool(name="qkp", bufs=8))
    vap = ctx.enter_context(tc.tile_pool(name="vap", bufs=16))
    ptp = ctx.enter_context(tc.tile_pool(name="ptp", bufs=3))
    otp = ctx.enter_context(tc.tile_pool(name="otp", bufs=16))
    rcp = ctx.enter_context(tc.tile_pool(name="rcp", bufs=2))
    xTp = ctx.enter_context(tc.tile_pool(name="xTp", bufs=8))
    wqkp = ctx.enter_context(tc.tile_pool(name="wqkp", bufs=4))
    wvp = ctx.enter_context(tc.tile_pool(name="wvp", bufs=1))
    wpp = ctx.enter_context(tc.tile_pool(name="wpp", bufs=1))
    ysp = ctx.enter_context(tc.tile_pool(name="ysp", bufs=3))

    # constants (bias DMAs issued after the first transpose batch below —
    # they are not needed until the first qkv unit completes)
    bqk_sb = const.tile([P, 8], f32)
    bvb = const.tile([P, GQ], f32)
    ones8 = const.tile([P, NHL, 1], f32)
    nc.vector.memset(ones8[:], 1.0)
    # tril causal mask, bf16: keep pt[p, j] where j >= p (q_local >= k_local)
    trilf = const.tile([P, P], f32)
    nc.vector.memset(trilf[:], 1.0)
    nc.gpsimd.affine_select(
        out=trilf[:], in_=trilf[:], compare_op=mybir.AluOpType.is_ge,
        fill=0.0, base=0, pattern=[[1, P]], channel_multiplier=-1)
    trilb = const.tile([P, P], bf16)
    nc.vector.tensor_copy(trilb[:], trilf[:])

    wqkv_r = wqkv_d.rearrange("(a p) n -> p a n", p=P)  # [128, 8, 1536]

    # ---------------- weight DMAs first (small, unblock qkv) ------------
    wqk_tiles = {}

    def load_wqk(ct, eng=None):
        w_t = wqkp.tile([P, NCHUNK, P], bf16, name=f"wqk_{ct}", tag="wqk")
        (eng or nc.sync).dma_start(w_t[:], wqkv_r[:, :, ct * P:ct * P + P])
        wqk_tiles[ct] = w_t

    # startup weights ride the ACT hwdge queue (idle until the first exp,
    # and plain 2D DMAs are safe there — only the transpose DMA corrupts)
    # so the serialized transpose stream below starts immediately.
    load_wqk(0, nc.scalar)
    load_wqk(4, nc.scalar)
    wv_t = wvp.tile([P, NCHUNK, GQ], bf16, name="wv", tag="wv")
    nc.scalar.dma_start(wv_t[:], wqkv_r[:, :, 2 * GQ:3 * GQ])

    # ---------------- xT via transpose DMA ----------------
    xT = []  # 8 tiles [128 c, 2048 t] bf16
    for r in range(NCHUNK):
        t_ = xTp.tile([P, T], bf16, name=f"xT{r}", tag="xT")
        xT.append(t_)
    # three batches: t 0:512 (unblocks pair-0 window 0 + V(0..3) fast),
    # t 512:1024, then t 1024:2048. All on the SP queue: ACT-issued
    # transpose DMAs corrupt data on HW. Small bias DMAs ride between
    # batches; wp (1 MB, needed only by pair 3) goes last.
    for t0, t1 in ((0, 512), (512, 1024), (1024, 2048)):
        for r in range(NCHUNK):
            nc.sync.dma_start_transpose(
                xT[r][:, t0:t1],
                x_d[t0:t1, r * P:(r + 1) * P],
            )
    nc.scalar.dma_start(bqk_sb[:], bqk_d[:])
    nc.scalar.dma_start(bvb[:], bv_d[None, :].to_broadcast((P, GQ)))
    wp_t = wpp.tile([P, 4, C], bf16, name="wp", tag="wp")
    nc.scalar.dma_start(wp_t[:], wp_d.rearrange("(a p) n -> p a n", p=P))

    # ---------------- qkv / proj emit units ----------------
    qkT = []  # bf16 tiles [128 c', 2048 t]; 0..3 = qT, 4..7 = kT
    for ct in range(8):
        o_t = qkp.tile([P, T], bf16, name=f"qkT{ct}", tag="qkT")
        qkT.append(o_t)

    vaug = []  # [128 k, 8 heads, 65] bf16 per k-chunk (col 64 = ones)
    for t in range(NT):
        va = vap.tile([P, NHL, HD + 1], bf16, name=f"vaug{t}", tag="vaug")
        nc.vector.tensor_copy(va[:, :, HD:HD + 1], ones8[:])
        vaug.append(va)

    def QK(ct, q, pieces=2):
        # one 512-wide quarter of qkT[ct], split into `pieces` chunks of the
        # 8-deep contraction; fillers own ps_fill so placement is free.
        st = {}
        step = NCHUNK // pieces

        def mk(pi):
            a0, a1 = pi * step, (pi + 1) * step

            def fn():
                if pi == 0:
                    if ct not in wqk_tiles:
                        load_wqk(ct)
                    st["ps"] = ps_fill.tile(
                        [P, 512], f32, name=f"qkps_{ct}_{q}", tag="fill")
                ps = st["ps"]
                for a in range(a0, a1):
                    nc.tensor.matmul(
                        ps[:], wqk_tiles[ct][:, a, :],
                        xT[a][:, q * 512:(q + 1) * 512],
                        start=(a == 0), stop=(a == NCHUNK - 1))
                if a1 == NCHUNK:
                    nc.vector.tensor_scalar_add(
                        qkT[ct][:, q * 512:(q + 1) * 512], ps[:],
                        bqk_sb[:, ct:ct + 1])
            return fn
        return [mk(pi) for pi in range(pieces)]

    def V(t, pieces=2):
        st = {}
        step = NCHUNK // pieces

        def mk(pi):
            a0, a1 = pi * step, (pi + 1) * step

            def fn():
                if pi == 0:
                    st["ps"] = ps_fill.tile(
                        [P, 512], f32, name=f"vps_{t}", tag="fill")
                ps = st["ps"]
                for a in range(a0, a1):
                    nc.tensor.matmul(
                        ps[:], xT[a][:, t * P:(t + 1) * P], wv_t[:, a, :],
                        start=(a == 0), stop=(a == NCHUNK - 1))
                if a1 == NCHUNK:
                    nc.vector.tensor_add(
                        vaug[t][:, :, 0:HD],
                        ps[:].rearrange("p (h d) -> p h d", h=NHL),
                        bvb[:].rearrange("p (h d) -> p h d", h=NHL))
            return fn
        return [mk(pi) for pi in range(pieces)]

    ot_all = {}  # (hp, m) -> [128, 512] bf16 tile in SBUF

    def PJ(m, mt):
        # one cout tile (128 rows of yT) for t window m; atomic (4 matmuls)
        def fn():
            ps = ps_fill.tile([P, 512], f32, name=f"yps_{m}_{mt}", tag="fill")
            for a in range(4):
                nc.tensor.matmul(
                    ps[:], wp_t[:, a, mt * P:(mt + 1) * P],
                    ot_all[(a, m)][:, :],
                    start=(a == 0), stop=(a == 3))
            ys = ysp.tile([P, 512], f32, name=f"ys_{m}_{mt}", tag="ys")
            nc.vector.tensor_copy(ys[:], ps[:])
            nc.sync.dma_start(
                yT_d[mt * P:(mt + 1) * P, m * 512:(m + 1) * 512], ys[:])
        return fn

    # ---------------- attention ----------------
    # Head pairs: head A on PE row strip 0, head B on strip 64; score pieces
    # for the two heads live in the two banks of one [128,1024] psum tile, so
    # the row-packed matmuls run concurrently and one exp covers both heads.
    # The PV accumulator is likewise one [128,1024] tile: head A cols 0:512,
    # head B cols 512:1024, partition 64 = denominators (ones column of vaug).
    def attn_pair(hp, sched):
        qt = qkT[hp]
        kt = qkT[4 + hp]
        for m in range(4):  # quarter windows of 512 q
            ws = m * 512
            pvt = ps_pv.tile([P, 1024], f32, name=f"pv_{hp}_{m}", tag="ps_pv")
            for i in range(4 * m + 4):  # causal k-chunks for this window
                s = max(i * P, ws)
                o = s - ws
                # head A piece in cols [o, 512), head B in [512, 1024-o)
                sc = ps_main.tile([P, 1024], f32, name=f"sc_{hp}_{m}_{i}",
                                  tag="main")
                for hh in range(2):
                    r0 = hh * HD
                    c0 = o if hh == 0 else 512
                    nc.tensor.matmul(
                        sc[:, c0:c0 + 512 - o],
                        kt[r0:r0 + HD, i * P:(i + 1) * P],
                        qt[r0:r0 + HD, s:ws + 512],
                        start=True,
                        stop=True,
                    )
                pt = ptp.tile([P, 1024], bf16, name=f"pt_{hp}_{m}_{i}",
                              tag="pt")
                nc.scalar.activation(pt[:, o:1024 - o], sc[:, o:1024 - o],
                                     EXP, scale=SCALE)
                # filler between the exp issue and the exp-dependent PV
                # matmuls: the in-order PE works through it while ACT
                # computes the exp, instead of stalling at PV.
                for fn in sched.get((m, i), ()):
                    fn()
                diag = i * P >= ws
                for hh in range(2):
                    c0 = o if hh == 0 else 512
                    if diag:
                        nc.vector.tensor_mul(
                            pt[:, c0:c0 + P], pt[:, c0:c0 + P], trilb[:])
                    nc.tensor.matmul(
                        pvt[0:HD + 1, hh * 512 + o:(hh + 1) * 512],
                        vaug[i][:, 2 * hp + hh, :],
                        pt[:, c0:c0 + 512 - o],
                        start=(i == 0),
                        stop=(i == 4 * m + 3),
                    )
            # normalize both heads at once: denominators to sbuf partition 0
            # (reciprocal_approx_fast mishandles nonzero partition offsets).
            # pvt is single-buffered, so evacuate it fast: the numerators are
            # copied out right after the denominators and the slow
            # recip/broadcast/mul tail then runs off sbuf.
            dn = rcp.tile([1, 1024], f32, name=f"dn_{hp}_{m}", tag="dn")
            nc.vector.tensor_copy(dn[:], pvt[HD:HD + 1, :])
            pvs = rcp.tile([HD, 1024], f32, name=f"pvs_{hp}_{m}", tag="pvs")
            nc.vector.tensor_copy(pvs[:], pvt[0:HD, :])
            rc = rcp.tile([1, 1024], f32, name=f"rc_{hp}_{m}", tag="rc")
            nc.vector.reciprocal_approx_fast(rc[:], dn[:])
            rcb = rcp.tile([HD, 1024], f32, name=f"rcb_{hp}_{m}", tag="rcb")
            nc.gpsimd.partition_broadcast(rcb[:], rc[:])
            ot = otp.tile([P, 512], bf16, name=f"ot_{hp}_{m}", tag="ot",
                          bufs=16)
            for hh in range(2):
                nc.vector.tensor_mul(
                    ot[hh * HD:(hh + 1) * HD, :],
                    pvs[:, hh * 512:(hh + 1) * 512],
                    rcb[:, hh * 512:(hh + 1) * 512])
            ot_all[(hp, m)] = ot

    # ---------------- schedule ----------------
    # pre-work: the minimum for pair 0 window 0 (qk quarters first so the
    # first scores fire ASAP; V(0..3) feed the first PVs)
    for ct in (0, 4):
        for f in QK(ct, 0):
            f()
    for t in range(4):
        for f in V(t):
            f()

    def mk():
        return {}

    def put(s, m, i, unit):
        # place unit pieces at consecutive chunks starting at (m, i)
        for k, f in enumerate(unit):
            s.setdefault((m, i + k), []).append(f)

    # pair 0: carries all remaining V units + its own q/k quarters + pair 1
    # q0/k0 — packed, so 2-piece units
    s0 = mk()
    put(s0, 0, 0, QK(0, 1))
    put(s0, 0, 2, V(4))       # due w1c4 (xT t 512:768 lands mid-window)
    put(s0, 0, 2, V(5))       # due w1c5
    put(s0, 1, 0, QK(4, 1))   # due w1c4
    put(s0, 1, 2, V(6))       # due w1c6
    put(s0, 1, 4, V(7))       # due w1c7
    put(s0, 1, 6, QK(0, 2))   # due w2c0
    put(s0, 2, 0, QK(4, 2))   # due w2c8
    put(s0, 2, 2, V(8))
    put(s0, 2, 4, V(9))
    put(s0, 2, 6, V(10))
    put(s0, 2, 8, V(11))
    put(s0, 2, 10, QK(0, 3))  # due w3c0
    put(s0, 3, 0, QK(4, 3))   # due w3c12
    put(s0, 3, 2, V(12))
    put(s0, 3, 4, V(13))
    put(s0, 3, 6, V(14))
    put(s0, 3, 8, V(15))
    put(s0, 3, 10, QK(1, 0))  # pair 1 w0
    put(s0, 3, 12, QK(5, 0))

    # pairs 1-3 have slack: 4-piece units spread the load evenly
    s1 = mk()
    put(s1, 0, 0, QK(1, 1))          # due p1w1c0; 2-piece (w0 is short)
    put(s1, 1, 0, QK(5, 1, 4))       # due w1c4
    put(s1, 1, 4, QK(1, 2, 4))       # due w2c0
    put(s1, 2, 0, QK(5, 2, 4))       # due w2c8
    put(s1, 2, 4, QK(1, 3, 4))
    put(s1, 2, 8, QK(2, 0, 4))
    put(s1, 3, 0, QK(5, 3, 4))       # due w3c12
    put(s1, 3, 4, QK(6, 0, 4))
    put(s1, 3, 8, QK(2, 1, 4))       # due p2w1c0

    s2 = mk()
    put(s2, 1, 0, QK(6, 1, 4))
    put(s2, 1, 4, QK(2, 2, 4))
    put(s2, 2, 0, QK(6, 2, 4))
    put(s2, 2, 4, QK(2, 3, 4))
    put(s2, 2, 8, QK(3, 0, 4))
    put(s2, 3, 0, QK(6, 3, 4))
    put(s2, 3, 4, QK(7, 0, 4))
    put(s2, 3, 8, QK(3, 1, 4))

    s3 = mk()
    put(s3, 1, 0, QK(7, 1, 4))       # due w1c4
    put(s3, 1, 4, QK(3, 2, 4))       # due w2c0
    put(s3, 1, 6, [PJ(0, 0), PJ(0, 1)])
    put(s3, 2, 0, QK(7, 2, 4))       # due w2c8
    put(s3, 2, 2, [PJ(0, 2), PJ(0, 3)])
    put(s3, 2, 4, QK(3, 3, 4))       # due w3c0
    put(s3, 2, 6, [PJ(0, 4), PJ(0, 5), PJ(0, 6), PJ(0, 7), PJ(1, 0),
                   PJ(1, 1)])
    put(s3, 3, 0, QK(7, 3, 4))       # due w3c12
    put(s3, 3, 4, [PJ(1, 2 + k) for k in range(6)])
    put(s3, 3, 10, [PJ(2, k) for k in range(6)])
    put(s3, 3, 14, [PJ(2, 6)])
    put(s3, 3, 15, [PJ(2, 7)])

    attn_pair(0, s0)
    attn_pair(1, s1)
    attn_pair(2, s2)
    attn_pair(3, s3)
    for mt in range(NCHUNK):
        PJ(3, mt)()


def _build_program():
    import contextlib

    import concourse.bass as bass
    import concourse.mybir as mybir
    import concourse.tile as tile
    from concourse import bacc

    nc = bacc.Bacc("TRN2", target_bir_lowering=False, debug=False, num_devices=8)
    f32 = mybir.dt.float32
    bf16 = mybir.dt.bfloat16
    aps = {
        "x": nc.dram_tensor("x", [T, C], bf16, kind="ExternalInput").ap(),
        "wqkv": nc.dram_tensor("wqkv", [C, 3 * GQ], bf16, kind="ExternalInput").ap(),
        "bqk": nc.dram_tensor("bqk", [P, 8], f32, kind="ExternalInput").ap(),
        "bv": nc.dram_tensor("bv", [GQ], f32, kind="ExternalInput").ap(),
        "wp": nc.dram_tensor("wp", [GQ, C], bf16, kind="ExternalInput").ap(),
        "yT": nc.dram_tensor("yT", [C, T], f32, kind="ExternalOutput").ap(),
    }
    with tile.TileContext(nc) as tc:
        with contextlib.ExitStack() as ctx:
            _emit(ctx, tc, aps, mybir, bass)
    nc.compile()
    return nc


def get_program():
    global _PROGRAM
    if _PROGRAM is None:
        _PROGRAM = _build_program()
    return _PROGRAM


def make_in_maps(x, w_qkv, b_qkv, w_proj):
    import ml_dtypes

    bf16 = ml_dtypes.bfloat16
    x = np.asarray(x, np.float32)
    w_qkv = np.asarray(w_qkv, np.float32)
    b_qkv = np.asarray(b_qkv, np.float32)
    w_proj = np.asarray(w_proj, np.float32)
    in_maps = []
    for c in range(8):
        b = c // 2
        g = c % 2
        q0 = g * GQ
        wq = w_qkv[:, q0:q0 + GQ]
        wk = w_qkv[:, C + q0:C + q0 + GQ]
        wv = w_qkv[:, 2 * C + q0:2 * C + q0 + GQ]
        wqkv = np.ascontiguousarray(
            np.concatenate([wq, wk, wv], axis=1).astype(bf16))
        bq = b_qkv[q0:q0 + GQ]
        bk = b_qkv[C + q0:C + q0 + GQ]
        bqk = np.ascontiguousarray(np.concatenate([bq, bk]).reshape(8, P).T)
        bv = np.ascontiguousarray(b_qkv[2 * C + q0:2 * C + q0 + GQ])
        in_maps.append({
            "x": np.ascontiguousarray(x[b].astype(bf16)),
            "wqkv": wqkv,
            "bqk": bqk,
            "bv": bv,
            "wp": np.ascontiguousarray(w_proj[q0:q0 + GQ, :].astype(bf16)),
        })
    return in_maps


def combine_outputs(outs, b_proj):
    b_proj = np.asarray(b_proj, np.float32)
    y = np.empty((B, T, C), np.float32)
    for b in range(B):
        acc = outs[2 * b] + outs[2 * b + 1]  # [C, T]
        y[b] = acc.T + b_proj
    return y


def kernel(x, w_qkv, b_qkv, w_proj, b_proj, _trace=False):
    from concourse import bass_utils

    nc = get_program()
    in_maps = make_in_maps(x, w_qkv, b_qkv, w_proj)
    res = bass_utils.run_bass_kernel_spmd(
        nc, in_maps, core_ids=list(range(8)), trace=_trace
    )
    outs = [r["yT"] for r in res.results]
    y = combine_outputs(outs, b_proj)
    if _trace:
        return y, res
    return y


# revision 27
# speedup vs baseline: 1.0021x; 1.0021x over previous
"""Causal self-attention on 8 TRN2 NeuronCores.

Sharding: core c handles batch b = c//2 and head-group g = c%2 (8 of 16 heads).
Each core computes its partial y^T = w_proj[slice].T @ o^T (contraction over its
512 o-channels); the host sums the two partials per batch and adds b_proj.

Shapes (hardcoded): B=4, T=2048, C=1024, H=16, HD=64.

All matmul operands are bf16 (x/w_qkv/w_proj cast on host); accumulation is
fp32 in PSUM. x^T is loaded straight from DRAM with the xbar transpose DMA
(issues split across the SP and ACT queues; weight DMAs issued first).
o stays in SBUF (bf16) and feeds proj directly.

Schedule: attention is ACT(exp)-bound, so qkv/v/proj work is emitted in
half-unit (4-matmul) chunks interleaved between attention chunks, keeping the
PE stream dense while ACT crunches exp without starving its 2-deep score
backlog. proj for window m runs inside pair 3 right after (3, m) completes.
Diagonal causal masking is a DVE multiply with a tril mask (gpsimd
affine_select is broken for bf16 on HW, and gpsimd cannot read PSUM).

PSUM (8 banks): ps_main 2x[128,1024] holds score tiles AND filler accumulators
(split filler halves interleave 1:1 with score allocs so rotation deps always
point backward); ps_pv 2x[128,1024] holds the per-window PV accumulator — both
heads side by side, so one reciprocal-normalize chain covers the window.
reciprocal_approx_fast needs its input at partition offset 0 (HW bug), hence
the denominator row is first copied to a [1,1024] sbuf tile.
"""

import numpy as np

B, T, C, H = 4, 2048, 1024, 16
HD = C // H          # 64
G = 2                # head groups
NHL = H // G         # 8 heads per core
GQ = NHL * HD        # 512 channel slice per core
P = 128
NT = T // P          # 16 token tiles / k-chunks
NCHUNK = C // P      # 8 contraction chunks for qkv
SCALE = 1.0 / float(np.sqrt(HD))

_PROGRAM = None


def _emit(ctx, tc, aps, mybir, bass):
    nc = tc.nc
    f32 = mybir.dt.float32
    bf16 = mybir.dt.bfloat16
    EXP = mybir.ActivationFunctionType.Exp

    x_d, wqk_d, wv_d, bqk_d, bv_d, wp_d, yT_d = (
        aps["x"], aps["wqk"], aps["wv"], aps["bqk"], aps["bv"], aps["wp"],
        aps["yT"],
    )

    # ---------------- pools ----------------
    const = ctx.enter_context(tc.tile_pool(name="const", bufs=1))
    ps_main = ctx.enter_context(tc.tile_pool(name="ps_main", bufs=2, space="PSUM"))
    ps_pv = ctx.enter_context(tc.tile_pool(name="ps_pv", bufs=1, space="PSUM"))
    ps_fill = ctx.enter_context(tc.tile_pool(name="ps_fill", bufs=2, space="PSUM"))

    qkp = ctx.enter_context(tc.tile_pool(name="qkp", bufs=8))
    vap = ctx.enter_context(tc.tile_pool(name="vap", bufs=16))
    ptp = ctx.enter_context(tc.tile_pool(name="ptp", bufs=3))
    otp = ctx.enter_context(tc.tile_pool(name="otp", bufs=16))
    rcp = ctx.enter_context(tc.tile_pool(name="rcp", bufs=2))
    xTp = ctx.enter_context(tc.tile_pool(name="xTp", bufs=8))
    wqkp = ctx.enter_context(tc.tile_pool(name="wqkp", bufs=4))
    wvp = ctx.enter_context(tc.tile_pool(name="wvp", bufs=1))
    wpp = ctx.enter_context(tc.tile_pool(name="wpp", bufs=1))
    ysp = ctx.enter_context(tc.tile_pool(name="ysp", bufs=3))

    # constants (bias DMAs issued after the first transpose batch below —
    # they are not needed until the first qkv unit completes)
    bqk_sb = const.tile([P, 8], f32)
    bvb = const.tile([P, GQ], f32)
    ones8 = const.tile([P, NHL, 1], f32)
    nc.vector.memset(ones8[:], 1.0)
    # tril causal mask, bf16: keep pt[p, j] where j >= p (q_local >= k_local)
    trilf = const.tile([P, P], f32)
    nc.vector.memset(trilf[:], 1.0)
    nc.gpsimd.affine_select(
        out=trilf[:], in_=trilf[:], compare_op=mybir.AluOpType.is_ge,
        fill=0.0, base=0, pattern=[[1, P]], channel_multiplier=-1)
    trilb = const.tile([P, P], bf16)
    nc.vector.tensor_copy(trilb[:], trilf[:])

    # ---------------- weight DMAs (host pre-arranged: contiguous rows) ---
    wqk_tiles = {}

    def load_wqk(ct, eng=None):
        w_t = wqkp.tile([P, NCHUNK, P], bf16, name=f"wqk_{ct}", tag="wqk")
        (eng or nc.sync).dma_start(w_t[:], wqk_d[ct])
        wqk_tiles[ct] = w_t

    # startup weights ride the ACT hwdge queue (idle until the first exp,
    # and plain 2D DMAs are safe there — only the transpose DMA corrupts)
    # so the serialized transpose stream below starts immediately.
    load_wqk(0, nc.scalar)
    load_wqk(4, nc.scalar)
    wv_t = wvp.tile([P, NCHUNK, GQ], bf16, name="wv", tag="wv")
    nc.scalar.dma_start(wv_t[:], wv_d[:])

    # ---------------- xT via transpose DMA ----------------
    xT = []  # 8 tiles [128 c, 2048 t] bf16
    for r in range(NCHUNK):
        t_ = xTp.tile([P, T], bf16, name=f"xT{r}", tag="xT")
        xT.append(t_)
    # three batches: t 0:512 (unblocks pair-0 window 0 + V(0..3) fast),
    # t 512:1024, then t 1024:2048. All on the SP queue: ACT-issued
    # transpose DMAs corrupt data on HW. Small bias DMAs ride between
    # batches; wp (1 MB, needed only by pair 3) goes last.
    for t0, t1 in ((0, 512), (512, 1024), (1024, 2048)):
        for r in range(NCHUNK):
            nc.sync.dma_start_transpose(
                xT[r][:, t0:t1],
                x_d[t0:t1, r * P:(r + 1) * P],
            )
    nc.scalar.dma_start(bqk_sb[:], bqk_d[:])
    nc.scalar.dma_start(bvb[:], bv_d[None, :].to_broadcast((P, GQ)))
    wp_t = wpp.tile([P, 4, C], bf16, name="wp", tag="wp")
    nc.scalar.dma_start(wp_t[:], wp_d[:])

    # ---------------- qkv / proj emit units ----------------
    qkT = []  # bf16 tiles [128 c', 2048 t]; 0..3 = qT, 4..7 = kT
    for ct in range(8):
        o_t = qkp.tile([P, T], bf16, name=f"qkT{ct}", tag="qkT")
        qkT.append(o_t)

    vaug = []  # [128 k, 8 heads, 65] bf16 per k-chunk (col 64 = ones)
    for t in range(NT):
        va = vap.tile([P, NHL, HD + 1], bf16, name=f"vaug{t}", tag="vaug")
        nc.vector.tensor_copy(va[:, :, HD:HD + 1], ones8[:])
        vaug.append(va)

    def QK(ct, q, pieces=2):
        # one 512-wide quarter of qkT[ct], split into `pieces` chunks of the
        # 8-deep contraction; fillers own ps_fill so placement is free.
        st = {}
        step = NCHUNK // pieces

        def mk(pi):
            a0, a1 = pi * step, (pi + 1) * step

            def fn():
                if pi == 0:
                    if ct not in wqk_tiles:
                        load_wqk(ct)
                    st["ps"] = ps_fill.tile(
                        [P, 512], f32, name=f"qkps_{ct}_{q}", tag="fill")
                ps = st["ps"]
                for a in range(a0, a1):
                    nc.tensor.matmul(
                        ps[:], wqk_tiles[ct][:, a, :],
                        xT[a][:, q * 512:(q + 1) * 512],
                        start=(a == 0), stop=(a == NCHUNK - 1))
                if a1 == NCHUNK:
                    nc.vector.tensor_scalar_add(
                        qkT[ct][:, q * 512:(q + 1) * 512], ps[:],
                        bqk_sb[:, ct:ct + 1])
            return fn
        return [mk(pi) for pi in range(pieces)]

    def V(t, pieces=2):
        st = {}
        step = NCHUNK // pieces

        def mk(pi):
            a0, a1 = pi * step, (pi + 1) * step

            def fn():
                if pi == 0:
                    st["ps"] = ps_fill.tile(
                        [P, 512], f32, name=f"vps_{t}", tag="fill")
                ps = st["ps"]
                for a in range(a0, a1):
                    nc.tensor.matmul(
                        ps[:], xT[a][:, t * P:(t + 1) * P], wv_t[:, a, :],
                        start=(a == 0), stop=(a == NCHUNK - 1))
                if a1 == NCHUNK:
                    nc.vector.tensor_add(
                        vaug[t][:, :, 0:HD],
                        ps[:].rearrange("p (h d) -> p h d", h=NHL),
                        bvb[:].rearrange("p (h d) -> p h d", h=NHL))
            return fn
        return [mk(pi) for pi in range(pieces)]

    ot_all = {}  # (hp, m) -> [128, 512] bf16 tile in SBUF

    def PJ(m, mt):
        # one cout tile (128 rows of yT) for t window m; atomic (4 matmuls)
        def fn():
            ps = ps_fill.tile([P, 512], f32, name=f"yps_{m}_{mt}", tag="fill")
            for a in range(4):
                nc.tensor.matmul(
                    ps[:], wp_t[:, a, mt * P:(mt + 1) * P],
                    ot_all[(a, m)][:, :],
                    start=(a == 0), stop=(a == 3))
            ys = ysp.tile([P, 512], f32, name=f"ys_{m}_{mt}", tag="ys")
            nc.vector.tensor_copy(ys[:], ps[:])
            nc.sync.dma_start(
                yT_d[mt * P:(mt + 1) * P, m * 512:(m + 1) * 512], ys[:])
        return fn

    # ---------------- attention ----------------
    # Head pairs: head A on PE row strip 0, head B on strip 64; score pieces
    # for the two heads live in the two banks of one [128,1024] psum tile, so
    # the row-packed matmuls run concurrently and one exp covers both heads.
    # The PV accumulator is likewise one [128,1024] tile: head A cols 0:512,
    # head B cols 512:1024, partition 64 = denominators (ones column of vaug).
    def attn_pair(hp, sched):
        qt = qkT[hp]
        kt = qkT[4 + hp]
        for m in range(4):  # quarter windows of 512 q
            ws = m * 512
            pvt = ps_pv.tile([P, 1024], f32, name=f"pv_{hp}_{m}", tag="ps_pv")
            for i in range(4 * m + 4):  # causal k-chunks for this window
                s = max(i * P, ws)
                o = s - ws
                # head A piece in cols [o, 512), head B in [512, 1024-o)
                sc = ps_main.tile([P, 1024], f32, name=f"sc_{hp}_{m}_{i}",
                                  tag="main")
                for hh in range(2):
                    r0 = hh * HD
                    c0 = o if hh == 0 else 512
                    nc.tensor.matmul(
                        sc[:, c0:c0 + 512 - o],
                        kt[r0:r0 + HD, i * P:(i + 1) * P],
                        qt[r0:r0 + HD, s:ws + 512],
                        start=True,
                        stop=True,
                    )
                pt = ptp.tile([P, 1024], bf16, name=f"pt_{hp}_{m}_{i}",
                              tag="pt")
                nc.scalar.activation(pt[:, o:1024 - o], sc[:, o:1024 - o],
                                     EXP, scale=SCALE)
                # filler between the exp issue and the exp-dependent PV
                # matmuls: the in-order PE works through it while ACT
                # computes the exp, instead of stalling at PV.
                for fn in sched.get((m, i), ()):
                    fn()
                diag = i * P >= ws
                for hh in range(2):
                    c0 = o if hh == 0 else 512
                    if diag:
                        nc.vector.tensor_mul(
                            pt[:, c0:c0 + P], pt[:, c0:c0 + P], trilb[:])
                    nc.tensor.matmul(
                        pvt[0:HD + 1, hh * 512 + o:(hh + 1) * 512],
                        vaug[i][:, 2 * hp + hh, :],
                        pt[:, c0:c0 + 512 - o],
                        start=(i == 0),
                        stop=(i == 4 * m + 3),
                    )
            # normalize both heads at once: denominators to sbuf partition 0
            # (reciprocal_approx_fast mishandles nonzero partition offsets).
            # pvt is single-buffered, so evacuate it fast: the numerators are
            # copied out right after the denominators and the slow
            # recip/broadcast/mul tail then runs off sbuf.
            dn = rcp.tile([1, 1024], f32, name=f"dn_{hp}_{m}", tag="dn")
            nc.vector.tensor_copy(dn[:], pvt[HD:HD + 1, :])
            pvs = rcp.tile([HD, 1024], f32, name=f"pvs_{hp}_{m}", tag="pvs")
            nc.vector.tensor_copy(pvs[:], pvt[0:HD, :])
            rc = rcp.tile([1, 1024], f32, name=f"rc_{hp}_{m}", tag="rc")
            nc.vector.reciprocal_approx_fast(rc[:], dn[:])
            rcb = rcp.tile([HD, 1024], f32, name=f"rcb_{hp}_{m}", tag="rcb")
            nc.gpsimd.partition_broadcast(rcb[:], rc[:])
            ot = otp.tile([P, 512], bf16, name=f"ot_{hp}_{m}", tag="ot",
                          bufs=16)
            for hh in range(2):
                nc.vector.tensor_mul(
                    ot[hh * HD:(hh + 1) * HD, :],
                    pvs[:, hh * 512:(hh + 1) * 512],
                    rcb[:, hh * 512:(hh + 1) * 512])
            ot_all[(hp, m)] = ot

    # ---------------- schedule ----------------
    # pre-work: the minimum for pair 0 window 0 (qk quarters first so the
    # first scores fire ASAP; V(0..3) feed the first PVs)
    for ct in (0, 4):
        for f in QK(ct, 0):
            f()
    for t in range(4):
        for f in V(t):
            f()

    def mk():
        return {}

    def put(s, m, i, unit):
        # place unit pieces at consecutive chunks starting at (m, i)
        for k, f in enumerate(unit):
            s.setdefault((m, i + k), []).append(f)

    # pair 0: carries all remaining V units + its own q/k quarters + pair 1
    # q0/k0 — packed, so 2-piece units
    s0 = mk()
    put(s0, 0, 0, QK(0, 1))
    put(s0, 0, 2, V(4))       # due w1c4 (xT t 512:768 lands mid-window)
    put(s0, 0, 2, V(5))       # due w1c5
    put(s0, 1, 0, QK(4, 1))   # due w1c4
    put(s0, 1, 2, V(6))       # due w1c6
    put(s0, 1, 4, V(7))       # due w1c7
    put(s0, 1, 6, QK(0, 2))   # due w2c0
    put(s0, 2, 0, QK(4, 2))   # due w2c8
    put(s0, 2, 2, V(8))
    put(s0, 2, 4, V(9))
    put(s0, 2, 6, V(10))
    put(s0, 2, 8, V(11))
    put(s0, 2, 10, QK(0, 3))  # due w3c0
    put(s0, 3, 0, QK(4, 3))   # due w3c12
    put(s0, 3, 2, V(12))
    put(s0, 3, 4, V(13))
    put(s0, 3, 6, V(14))
    put(s0, 3, 8, V(15))
    put(s0, 3, 10, QK(1, 0))  # pair 1 w0
    put(s0, 3, 12, QK(5, 0))

    # pairs 1-3 have slack: 4-piece units spread the load evenly
    s1 = mk()
    put(s1, 0, 0, QK(1, 1))          # due p1w1c0; 2-piece (w0 is short)
    put(s1, 1, 0, QK(5, 1, 4))       # due w1c4
    put(s1, 1, 4, QK(1, 2, 4))       # due w2c0
    put(s1, 2, 0, QK(5, 2, 4))       # due w2c8
    put(s1, 2, 4, QK(1, 3, 4))
    put(s1, 2, 8, QK(2, 0, 4))
    put(s1, 3, 0, QK(5, 3, 4))       # due w3c12
    put(s1, 3, 4, QK(6, 0, 4))
    put(s1, 3, 8, QK(2, 1, 4))       # due p2w1c0

    s2 = mk()
    put(s2, 1, 0, QK(6, 1, 4))
    put(s2, 1, 4, QK(2, 2, 4))
    put(s2, 2, 0, QK(6, 2, 4))
    put(s2, 2, 4, QK(2, 3, 4))
    put(s2, 2, 8, QK(3, 0, 4))
    put(s2, 3, 0, QK(6, 3, 4))
    put(s2, 3, 4, QK(7, 0, 4))
    put(s2, 3, 8, QK(3, 1, 4))

    s3 = mk()
    put(s3, 1, 0, QK(7, 1, 4))       # due w1c4
    put(s3, 1, 4, QK(3, 2, 4))       # due w2c0
    put(s3, 1, 6, [PJ(0, 0), PJ(0, 1)])
    put(s3, 2, 0, QK(7, 2, 4))       # due w2c8
    put(s3, 2, 2, [PJ(0, 2), PJ(0, 3)])
    put(s3, 2, 4, QK(3, 3, 4))       # due w3c0
    put(s3, 2, 6, [PJ(0, 4), PJ(0, 5), PJ(0, 6), PJ(0, 7), PJ(1, 0),
                   PJ(1, 1)])
    put(s3, 3, 0, QK(7, 3, 4))       # due w3c12
    put(s3, 3, 4, [PJ(1, 2 + k) for k in range(6)])
    put(s3, 3, 10, [PJ(2, k) for k in range(6)])
    put(s3, 3, 14, [PJ(2, 6)])
    put(s3, 3, 15, [PJ(2, 7)])

    attn_pair(0, s0)
    attn_pair(1, s1)
    attn_pair(2, s2)
    attn_pair(3, s3)
    for mt in range(NCHUNK):
        PJ(3, mt)()


def _build_program():
    import contextlib

    import concourse.bass as bass
    import concourse.mybir as mybir
    import concourse.tile as tile
    from concourse import bacc

    nc = bacc.Bacc("TRN2", target_bir_lowering=False, debug=False, num_devices=8)
    f32 = mybir.dt.float32
    bf16 = mybir.dt.bfloat16
    aps = {
        "x": nc.dram_tensor("x", [T, C], bf16, kind="ExternalInput").ap(),
        # weights pre-arranged on host for contiguous per-partition loads:
        # wqk[ct, p, a*128+j] = w_qkv[a*128+p, (q|k slice) ct*128+j]
        "wqk": nc.dram_tensor("wqk", [8, P, NCHUNK * P], bf16,
                              kind="ExternalInput").ap(),
        # wv[p, a, j] = w_qkv[a*128+p, v-slice j]
        "wv": nc.dram_tensor("wv", [P, NCHUNK, GQ], bf16,
                             kind="ExternalInput").ap(),
        "bqk": nc.dram_tensor("bqk", [P, 8], f32, kind="ExternalInput").ap(),
        "bv": nc.dram_tensor("bv", [GQ], f32, kind="ExternalInput").ap(),
        # wp[p, a, j] = w_proj[a*128+p (in gq slice), j]
        "wp": nc.dram_tensor("wp", [P, 4, C], bf16, kind="ExternalInput").ap(),
        "yT": nc.dram_tensor("yT", [C, T], f32, kind="ExternalOutput").ap(),
    }
    with tile.TileContext(nc) as tc:
        with contextlib.ExitStack() as ctx:
            _emit(ctx, tc, aps, mybir, bass)
    nc.compile()
    return nc


def get_program():
    global _PROGRAM
    if _PROGRAM is None:
        _PROGRAM = _build_program()
    return _PROGRAM


def make_in_maps(x, w_qkv, b_qkv, w_proj):
    import ml_dtypes

    bf16 = ml_dtypes.bfloat16
    x = np.asarray(x, np.float32)
    w_qkv = np.asarray(w_qkv, np.float32)
    b_qkv = np.asarray(b_qkv, np.float32)
    w_proj = np.asarray(w_proj, np.float32)
    in_maps = []
    for c in range(8):
        b = c // 2
        g = c % 2
        q0 = g * GQ
        wq = w_qkv[:, q0:q0 + GQ]
        wk = w_qkv[:, C + q0:C + q0 + GQ]
        wv = w_qkv[:, 2 * C + q0:2 * C + q0 + GQ]
        # wqk[ct, p, a*128+j] = qk[a*128+p, ct*128+j] where qk = [wq | wk]
        qk = np.concatenate([wq, wk], axis=1)        # [C, 1024]
        wqk = qk.reshape(NCHUNK, P, 8, P).transpose(2, 1, 0, 3).reshape(
            8, P, NCHUNK * P)
        # wv_r[p, a, j] = wv[a*128+p, j]
        wv_r = wv.reshape(NCHUNK, P, GQ).transpose(1, 0, 2)
        # wp_r[p, a, j] = w_proj[q0 + a*128+p, j]
        wp_r = w_proj[q0:q0 + GQ, :].reshape(4, P, C).transpose(1, 0, 2)
        bq = b_qkv[q0:q0 + GQ]
        bk = b_qkv[C + q0:C + q0 + GQ]
        bqk = np.ascontiguousarray(np.concatenate([bq, bk]).reshape(8, P).T)
        bv = np.ascontiguousarray(b_qkv[2 * C + q0:2 * C + q0 + GQ])
        in_maps.append({
            "x": np.ascontiguousarray(x[b].astype(bf16)),
            "wqk": np.ascontiguousarray(wqk.astype(bf16)),
            "wv": np.ascontiguousarray(wv_r.astype(bf16)),
            "bqk": bqk,
            "bv": bv,
            "wp": np.ascontiguousarray(wp_r.astype(bf16)),
        })
    return in_maps


def combine_outputs(outs, b_proj):
    b_proj = np.asarray(b_proj, np.float32)
    y = np.empty((B, T, C), np.float32)
    for b in range(B):
        acc = outs[2 * b] + outs[2 * b + 1]  # [C, T]
        y[b] = acc.T + b_proj
    return y


def kernel(x, w_qkv, b_qkv, w_proj, b_proj, _trace=False):
    from concourse import bass_utils

    nc = get_program()
    in_maps = make_in_maps(x, w_qkv, b_qkv, w_proj)
    res = bass_utils.run_bass_kernel_spmd(
        nc, in_maps, core_ids=list(range(8)), trace=_trace
    )
    outs = [r["yT"] for r in res.results]
    y = combine_outputs(outs, b_proj)
    if _trace:
        return y, res
    return y
